# revision 17
# baseline (speedup 1.0000x reference)
"""Trainium2 Bass kernel for a 6-layer geometry-constrained cross-attention decoder.

Sharding: pure data-parallel over batch B=8 -> one batch element per NeuronCore.
Per-core layouts are feature-major ("T" = transposed): activations live as
[feature, token].

Fully fp8-DoubleRow matmul pipeline (0.5 PE-cycles per output row):
- CA/SA attention q/k/scores and probabilities are fp8 end to end.
- The geometry mask is applied on the PE: an fp8 identity matmul accumulates
  a {0, -176} mask bias into the scores PSUM ahead of the exp, so the former
  per-group DVE mask multiply disappears entirely.
- Softmax exp emits fp8 probabilities straight from the Act engine (free);
  AV contracts 256 keys per DR pass against fp8 V (ones rows in the V tile
  produce the softmax denominator in the same pass).
- FFN runs fp8-DR end to end; weights are scaled x32 into e4m3's normal
  range and de-scaled inside the bias/relu stages. The FFN2 output bias is
  folded in as an extra contraction pair against a persistent ones vector.
- LayerNorm rstd = exp(-0.5*ln(var+eps)); ln/exp share one activation table
  (compile-time table hint) so the Act engine never reloads tables.
- The next layer's k-projection is interleaved into the CA attention heads'
  PE slack; the v-projection overlaps the SA attention phase.

Residual stream, layernorm statistics, biases and PSUM accumulation in fp32.
"""

import os
import sys

for _p in ("/opt/trn_rl_repo", "/root/.axon_site/_ro/trn_rl_repo"):
    if os.path.isdir(_p) and _p not in sys.path:
        sys.path.insert(0, _p)

import numpy as np
import ml_dtypes

import concourse.bass as bass
import concourse.tile as tile
from concourse import bacc
from concourse import mybir
from concourse import bass_utils

BF16 = ml_dtypes.bfloat16
FP8 = ml_dtypes.float8_e4m3
F32 = np.float32

B, NQ, NK, E, H, F, L = 8, 300, 4096, 256, 8, 2048, 6
D = E // H
SCALE = D ** -0.5
PC = 128          # partitions
EC = E // PC      # 2 feature chunks
FT = F // PC      # 16 ffn chunks
KT_CA = NK // PC  # 32 cross-attention key tiles
KP_CA = KT_CA // 2  # 16 DR key-tile pairs
TOK_TILES = [(0, 100), (100, 100), (200, 100)]   # 300 tokens, uniform
WS = 32.0         # fp8 weight scaling (into e4m3 normal range)
MASKB = -176.0    # additive mask bias (exp(SCALE*-176) ~ 5e-14)

dt = mybir.dt
Alu = mybir.AluOpType
Act = mybir.ActivationFunctionType
DR = mybir.MatmulPerfMode.DoubleRow

# smalls column map (per-partition fp32 vectors, feature f = 128*c + p)
C_BQ_SA = 0   # 2 cols: sa q bias
C_BO_SA = 4   # 2 (includes folded sa v-bias)
C_BQ_CA = 6   # 2
C_BO_CA = 8   # 2 (includes folded ca v-bias)
C_B1 = 12     # 16 (x WS)
C_LN = 30     # 12: ln1g ln1b ln2g ln2b ln3g ln3b (2 each)
NS = 42


def _bcmid(ap2d, c):
    """[P, N] AP -> [P, c, N] with the middle dim broadcast (step 0)."""
    return bass.AP(tensor=ap2d.tensor, offset=ap2d.offset,
                   ap=[list(ap2d.ap[0]), [0, c], list(ap2d.ap[-1])])


def _patch_act_tables():
    """Compile-time hint: make Exp/Ln resolve to the one table set that
    contains both ('natural_log_exp_and_others'), so the greedy table-load
    pass emits a single load instead of thrashing between sets. Set ids and
    contents seen by the NEFF compiler are unchanged."""
    from concourse import hw_specs as _hw
    from concourse import bacc as _bacc
    if getattr(_hw, "_act_tables_patched", False):
        return
    orig = _hw.get_activation_tables

    def patched(arch):
        t = dict(orig(arch))
        A = mybir.ActivationFunctionType
        keep = "natural_log_exp_and_others"
        if keep in t and A.Exp in t[keep] and A.Ln in t[keep]:
            t = {name: (funcs if name == keep else funcs - {A.Exp, A.Ln})
                 for name, funcs in t.items()}
        return t

    _hw.get_activation_tables = patched
    _hw._act_tables_patched = True
    if getattr(_bacc, "get_activation_tables", None) is orig:
        _bacc.get_activation_tables = patched


def build_nc(nlayers=L):
    _patch_act_tables()
    nc = bacc.Bacc("TRN2", target_bir_lowering=False, debug=False)
    f32, bf, f8 = dt.float32, dt.bfloat16, dt.float8e4

    def din(name, shape, d=bf):
        return nc.dram_tensor(name, shape, d, kind="ExternalInput").ap()

    d_tT = din("tT", [E, NQ], f32)
    d_memT8 = din("memT8", [E, NK], f8)
    d_maskb8 = din("maskb8", [64, KT_CA, 2, NQ], f8)
    d_ident8 = din("ident8", [64, 2, PC], f8)
    d_wqk8 = din("w_sa_qk8", [nlayers, E, 2 * E], f8)
    d_wsv8 = din("w_sa_v8", [nlayers, E, E], f8)
    d_wso = din("w_sa_o", [nlayers, E, E])
    d_wcq8 = din("w_ca_q8", [nlayers, E, E], f8)
    d_wck8 = din("w_ca_k8", [nlayers, E, E], f8)
    d_wcv8 = din("w_ca_v8", [nlayers, E, E], f8)
    d_wco = din("w_ca_o", [nlayers, E, E])
    d_w18 = din("w_f18", [nlayers, E, 2 * F], f8)
    d_w28 = din("w_f28", [nlayers, 17 * PC, E])
    d_sm = din("smalls", [nlayers, PC, NS], f32)
    d_fin = din("finals", [PC, 4], f32)
    d_out = nc.dram_tensor("outT", [E, NQ], f32, kind="ExternalOutput").ap()

    def r2(ap):  # [256, X] -> [128, 2, X]
        return ap.rearrange("(c p) o -> p c o", p=PC)

    with tile.TileContext(nc) as tc:
        with (
            tc.tile_pool(name="persist", bufs=1) as pst,
            tc.tile_pool(name="wts", bufs=2) as wp,
            tc.tile_pool(name="acts", bufs=2) as acts,
            tc.tile_pool(name="probs", bufs=6) as probs,
            tc.tile_pool(name="stats", bufs=2) as stp,
            tc.tile_pool(name="ps_sc", bufs=2, space="PSUM") as ps_sc,
            tc.tile_pool(name="ps_pp", bufs=2, space="PSUM") as ps_pp,
        ):
            # ---- persistent loads (memT8/maskb8 queued after tT: they are
            # only needed from the CA phase on, tT feeds layer 0's SA) ----
            memT8 = pst.tile([PC, EC, NK], f8, tag="memT8", name="memT8_sb")
            maskb8 = pst.tile([64, KT_CA, 2, NQ], f8, tag="maskb8", name="maskb8_sb")
            ident8 = pst.tile([64, 2, PC], f8, tag="ident8", name="ident8_sb")
            eps = pst.tile([PC, 1], f32, tag="eps", name="eps_sb")
            nc.vector.memset(eps, 1e-5)
            ones = pst.tile([PC, PC], bf, tag="ones", name="ones_sb")
            nc.vector.memset(ones, 1.0)
            fin = pst.tile([PC, 4], f32, tag="fin", name="fin_sb")
            nc.sync.dma_start(out=fin, in_=d_fin)
            honk = pst.tile([PC, NQ], bf, tag="honk", name="honk_sb")
            nc.gpsimd.memset(honk, 1.0)
            vsa = pst.tile([PC, 3, H, 2 * D], f8, tag="vsa", name="vsa_sb")
            nc.gpsimd.memset(vsa[:, :, :, D:2 * D], 1.0)
            vca = pst.tile([PC, KT_CA, H, 2 * D], f8, tag="vca", name="vca_sb")
            nc.gpsimd.memset(vca[:, :, :, D:2 * D], 1.0)

            tT = acts.tile([PC, EC, NQ], f32, tag="tT", name="tT0")
            nc.sync.dma_start(out=tT, in_=r2(d_tT))
            tb8 = acts.tile([PC, EC, NQ], f8, tag="tb8", name="tb8_0")
            nc.gpsimd.tensor_copy(out=tb8, in_=tT)
            nc.sync.dma_start(out=memT8, in_=r2(d_memT8))
            nc.sync.dma_start(out=ident8, in_=d_ident8)

            def layernorm(l, r, gcol, name, emit=True, emit_resid=False):
                """r: [128, 2, 300] f32 -> (t_new f32, tb8_new fp8-or-None)"""
                rb = acts.tile([PC, EC, NQ], bf, tag="rb", name=f"rb{name}", bufs=1)
                nc.vector.tensor_copy(out=rb, in_=r)
                tsq = acts.tile([PC, EC, NQ], bf, tag="tsq", name=f"tsq{name}", bufs=1)
                nc.vector.tensor_mul(out=tsq, in0=rb, in1=rb)
                s0 = ps_pp.tile([PC, 512], f32, tag="pp", name=f"lns0{name}")
                s1 = ps_pp.tile([PC, 512], f32, tag="pp", name=f"lns1{name}")
                for c in range(EC):
                    nc.tensor.matmul(out=s0[:, 0:NQ], lhsT=ones,
                                     rhs=rb[:, c, :],
                                     start=(c == 0), stop=(c == EC - 1))
                for c in range(EC):
                    nc.tensor.matmul(out=s1[:, 0:NQ], lhsT=ones,
                                     rhs=tsq[:, c, :],
                                     start=(c == 0), stop=(c == EC - 1))
                # stats chain stays on one engine (DVE) in dependency order so
                # the Act ln/exp can start as early as possible; c1 follows.
                mean = stp.tile([PC, NQ], f32, tag="mean", name=f"mean{name}", bufs=1)
                nc.vector.tensor_scalar_mul(out=mean, in0=s0[:, 0:NQ], scalar1=1.0 / E)
                msq = stp.tile([PC, NQ], f32, tag="msq", name=f"msq{name}", bufs=1)
                nc.vector.tensor_mul(out=msq, in0=mean, in1=mean)
                var = stp.tile([PC, NQ], f32, tag="var", name=f"var{name}", bufs=1)
                nc.vector.scalar_tensor_tensor(out=var, in0=s1[:, 0:NQ], scalar=1.0 / E,
                                               in1=msq, op0=Alu.mult, op1=Alu.subtract)
                # rstd = (var + eps)^-0.5 via ln/exp (same act table as Exp)
                lnv = stp.tile([PC, NQ], f32, tag="lnv", name=f"lnv{name}", bufs=1)
                nc.scalar.activation(out=lnv, in_=var, func=Act.Ln, bias=eps[:, 0:1])
                rstd = stp.tile([PC, NQ], f32, tag="rstd", name=f"rstd{name}", bufs=1)
                nc.scalar.activation(out=rstd, in_=lnv, func=Act.Exp, scale=-0.5)
                c1 = acts.tile([PC, EC, NQ], f32, tag="c1", name=f"c1{name}", bufs=1)
                nc.vector.tensor_sub(out=c1, in0=r, in1=_bcmid(mean, EC))
                c2 = acts.tile([PC, EC, NQ], f32, tag="c2", name=f"c2{name}", bufs=1)
                nc.vector.tensor_mul(out=c2, in0=c1, in1=_bcmid(rstd, EC))
                t_new = acts.tile([PC, EC, NQ], f32, tag="tT", name=f"t{name}")
                if gcol is None:
                    g, b = fin[:, 0:2], fin[:, 2:4]
                else:
                    g = sm[:, gcol:gcol + 2]
                    b = sm[:, gcol + 2:gcol + 4]
                tb8_new = None
                if emit:
                    tb8_new = acts.tile([PC, EC, NQ], f8, tag="tb8", name=f"tb{name}")
                for c in range(EC):
                    if emit:
                        nc.vector.tensor_scalar(out=tb8_new[:, c, :], in0=c2[:, c, :],
                                                scalar1=g[:, c:c + 1], scalar2=b[:, c:c + 1],
                                                op0=Alu.mult, op1=Alu.add)
                    nc.gpsimd.tensor_scalar(out=t_new[:, c, :], in0=c2[:, c, :],
                                            scalar1=g[:, c:c + 1], scalar2=b[:, c:c + 1],
                                            op0=Alu.mult, op1=Alu.add)
                tb8_res = None
                if emit_resid:
                    # fp8 error-feedback residual of the emit (for FFN1)
                    tb8_res = acts.tile([PC, EC, NQ], f8, tag="tb8r", name=f"tbr{name}")
                    nc.vector.scalar_tensor_tensor(
                        out=tb8_res, in0=tb8_new, scalar=-1.0, in1=t_new,
                        op0=Alu.mult, op1=Alu.add)
                return t_new, tb8_new, tb8_res

            def sa_attention(q8, k8, name, per_head_emit=None):
                """SA fp8 attention. q8/k8 [128, 2, 300] feature-major; vsa
                [128(100), 3, H, 64] fp8; returns attn [128, 2, 300] bf16.
                Scores for head h+1 are emitted before head h's AV so the
                exps run back-to-back."""
                attn = acts.tile([PC, EC, NQ], bf, tag="attn", name=f"attn{name}")
                nkt = len(TOK_TILES)

                def emit_sc(h):
                    po = 32 * (h % 4)
                    ci = h // 4
                    sc = ps_sc.tile([PC, 3, 512], f32, tag="sc", name=f"sc{name}h{h}")
                    for j in range(nkt):
                        kt0, ksz = TOK_TILES[j]
                        nc.tensor.matmul(
                            out=sc[0:ksz, j, 0:NQ],
                            lhsT=k8[po:po + 32, ci, kt0:kt0 + ksz],
                            rhs=q8[po:po + 32, ci, 0:NQ],
                            start=True, stop=True,
                            tile_position=(po, 0))
                    return sc

                sc = emit_sc(0)
                for h in range(H):
                    po = 32 * (h % 4)
                    ci = h // 4
                    av = ps_pp.tile([PC, 512], f32, tag="pp", name=f"av{name}h{h}")
                    p8 = probs.tile([PC, 3, NQ], f8, tag="p",
                                    name=f"p{name}h{h}", bufs=10)
                    nc.scalar.activation(out=p8[0:100, 0:3, :],
                                         in_=sc[0:100, 0:3, 0:NQ], func=Act.Exp,
                                         scale=SCALE)
                    if h + 1 < H:
                        sc = emit_sc(h + 1)
                    if per_head_emit is not None:
                        per_head_emit(h)
                    nc.tensor.matmul(
                        out=av[0:2 * D, 0:NQ],
                        lhsT=vsa[0:100, 0:2, h, 0:2 * D],
                        rhs=p8[0:100, 0:2, 0:NQ],
                        start=True, stop=False, perf_mode=DR,
                        skip_group_check=True)
                    nc.tensor.matmul(
                        out=av[0:2 * D, 0:NQ],
                        lhsT=vsa[0:100, 2, h, 0:2 * D],
                        rhs=p8[0:100, 2, 0:NQ],
                        start=False, stop=True,
                        skip_group_check=True)
                    recip = stp.tile([32, NQ], f32, tag="recip",
                                     name=f"rc{name}h{h}", bufs=4)
                    nc.vector.reciprocal(out=recip, in_=av[D:2 * D, 0:NQ])
                    nc.vector.tensor_mul(out=attn[po:po + 32, ci, :],
                                         in0=av[0:D, 0:NQ], in1=recip)
                return attn

            def ca_attention(l, q8, wckT8, name, bg_emit=None, bg_at=10,
                             per_head_emit=None):
                """k-projection fused into the scores: per head,
                qw_h = fp8(16 * Wk_h^T q_h); then 16*s_h = memT8^T qw_h as an
                fp8-DR matmul over the 256 memory features (the k-tensor is
                never materialized). The identity mask-add carries value 16 so
                the {0,-176} bias lands in the 16x-scaled PSUM; exp applies
                SCALE/16. fp8-DR AV with ones-rows for the denominator."""
                attn = acts.tile([PC, EC, NQ], bf, tag="attn", name=f"attn{name}")
                qws = {}

                def emit_qw(h):
                    po = 32 * (h % 4)
                    ci = h // 4
                    qwp = ps_sc.tile([PC, 3, 512], f32, tag="sc",
                                     name=f"qwp{name}h{h}")
                    for ec in range(EC):
                        nc.tensor.matmul(
                            out=qwp[:, ec, 0:NQ],
                            lhsT=wckT8[po:po + 32, ci, PC * ec:PC * (ec + 1)],
                            rhs=q8[po:po + 32, ci, 0:NQ],
                            start=True, stop=True,
                            tile_position=(po, 0))
                    qw8 = probs.tile([PC, EC, NQ], f8, tag="qw",
                                     name=f"qw{name}h{h}", bufs=3)
                    nc.vector.tensor_scalar_mul(out=qw8, in0=qwp[:, 0:EC, 0:NQ],
                                                scalar1=0.5)
                    qws[h] = qw8
                groups = []
                g = 0
                while g < KT_CA:
                    groups.append((g, min(3, KT_CA - g)))
                    g += groups[-1][1]
                NG = len(groups)
                tasks = [(h, gi) for h in range(H) for gi in range(NG)]
                avs = {}

                def emit_sc(h, gi):
                    g0, gsz = groups[gi]
                    sc = ps_sc.tile([PC, 3, 512], f32, tag="sc",
                                    name=f"sc{name}h{h}g{g0}")
                    for j in range(gsz):
                        kt = g0 + j
                        nc.tensor.matmul(
                            out=sc[0:PC, j, 0:NQ],
                            lhsT=memT8[:, :, PC * kt:PC * (kt + 1)],
                            rhs=qws[h],
                            start=True, stop=False, perf_mode=DR,
                            skip_group_check=True)
                        nc.tensor.matmul(
                            out=sc[0:PC, j, 0:NQ],
                            lhsT=ident8,
                            rhs=maskb8[:, kt, :, :],
                            start=False, stop=True, perf_mode=DR,
                            skip_group_check=True)
                    return sc

                def emit_av(h, g0, gsz, p8):
                    # DR over the leading pair, single pass for the tail tile
                    if gsz >= 2:
                        nc.tensor.matmul(
                            out=avs[h][0:2 * D, 0:NQ],
                            lhsT=vca[:, g0:g0 + 2, h, 0:2 * D],
                            rhs=p8[:, 0:2, 0:NQ],
                            start=(g0 == 0), stop=(g0 + gsz == KT_CA and gsz == 2),
                            perf_mode=DR, skip_group_check=True)
                    if gsz != 2:
                        j = gsz - 1
                        nc.tensor.matmul(
                            out=avs[h][0:2 * D, 0:NQ],
                            lhsT=vca[:, g0 + j, h, 0:2 * D],
                            rhs=p8[:, j, 0:NQ],
                            start=(g0 == 0 and gsz == 1), stop=(g0 + gsz == KT_CA),
                            skip_group_check=True)

                def finish_head(h):
                    po = 32 * (h % 4)
                    ci = h // 4
                    recip = stp.tile([32, NQ], f32, tag="recip",
                                     name=f"rc{name}h{h}", bufs=4)
                    nc.vector.reciprocal(out=recip, in_=avs[h][D:2 * D, 0:NQ])
                    nc.vector.tensor_mul(out=attn[po:po + 32, ci, :],
                                         in0=avs[h][0:D, 0:NQ], in1=recip)

                # flat (head, pair) pipeline: exp for task i, scores for task
                # i+1, then the (lagged) AV of task i-1 — so neither a head
                # boundary nor the exp ever head-of-line blocks the streams.
                emit_qw(0)
                emit_qw(1)
                sc = emit_sc(0, 0)
                pend = None
                for idx, (h, gi) in enumerate(tasks):
                    g0, gsz = groups[gi]
                    if gi == 0:
                        avs[h] = ps_pp.tile([PC, 512], f32, tag="pp",
                                            name=f"av{name}h{h}")
                    p8 = probs.tile([PC, 3, NQ], f8, tag="p",
                                    name=f"p{name}h{h}g{g0}", bufs=10)
                    nc.scalar.activation(out=p8[:, 0:gsz, :],
                                         in_=sc[:, 0:gsz, 0:NQ], func=Act.Exp,
                                         scale=SCALE / 16.0)
                    if idx + 1 < len(tasks):
                        sc = emit_sc(*tasks[idx + 1])
                    # the rest of the v-projection must be in the PE stream
                    # before any AV matmul that reads vca[2*bg_at:]
                    if h == 0 and bg_emit is not None and g0 + gsz > 2 * bg_at - 3:
                        bg_emit()
                        bg_emit = None
                    if pend is not None:
                        ph, pg0, pgsz, pp8 = pend
                        emit_av(ph, pg0, pgsz, pp8)
                        if pg0 + pgsz == KT_CA:
                            finish_head(ph)
                    if gi == 5:
                        if h + 2 < H:
                            emit_qw(h + 2)
                        if per_head_emit is not None:
                            per_head_emit(h)   # mid-head: away from the boundary
                    pend = (h, g0, gsz, p8)
                ph, pg0, pgsz, pp8 = pend
                emit_av(ph, pg0, pgsz, pp8)
                finish_head(ph)
                return attn

            def out_proj_residual(l, w_sb, attn, bcol, tT, name):
                r = acts.tile([PC, EC, NQ], f32, tag="r", name=f"r{name}", bufs=1)
                pos = ps_sc.tile([PC, 3, 512], f32, tag="sc", name=f"po{name}")
                for co in range(EC):
                    for ci in range(EC):
                        nc.tensor.matmul(out=pos[:, co, 0:NQ],
                                         lhsT=w_sb[:, ci, PC * co:PC * (co + 1)],
                                         rhs=attn[:, ci, :],
                                         start=(ci == 0), stop=(ci == EC - 1))
                for co in range(EC):
                    nc.vector.scalar_tensor_tensor(
                        out=r[:, co, :], in0=pos[:, co, 0:NQ],
                        scalar=sm[:, bcol + co:bcol + co + 1],
                        in1=tT[:, co, :], op0=Alu.add, op1=Alu.add)
                return r

            wsm_next = None
            for l in range(nlayers):
                # ---- layer weight loads (smalls first: the first SA
                # bias op waits on it) ----
                if l == 0:
                    sm = wp.tile([PC, NS], f32, tag="sm", name=f"sm{l}")
                    nc.sync.dma_start(out=sm, in_=d_sm[l])
                wqk8 = wp.tile([PC, EC, 2 * E], f8, tag="wqk", name=f"wqk{l}")
                nc.sync.dma_start(out=wqk8, in_=r2(d_wqk8[l]))
                wsv8 = wp.tile([PC, EC, E], f8, tag="wsv", name=f"wsv{l}")
                nc.sync.dma_start(out=wsv8, in_=r2(d_wsv8[l]))
                wso = wp.tile([PC, EC, E], bf, tag="wso", name=f"wso{l}")
                nc.sync.dma_start(out=wso, in_=r2(d_wso[l]))

                wcq8 = wp.tile([PC, EC, E], f8, tag="wcq8", name=f"wcq8{l}")
                nc.sync.dma_start(out=wcq8, in_=r2(d_wcq8[l]))
                wckT8 = wp.tile([PC, EC, E], f8, tag="wck8", name=f"wck8{l}")
                nc.sync.dma_start(out=wckT8, in_=r2(d_wck8[l]))
                wcv8 = wp.tile([PC, EC, E], f8, tag="wcv8", name=f"wcv8{l}")
                nc.sync.dma_start(out=wcv8, in_=r2(d_wcv8[l]))
                wco = wp.tile([PC, EC, E], bf, tag="wco", name=f"wco{l}")
                nc.sync.dma_start(out=wco, in_=r2(d_wco[l]))
                w18 = wp.tile([PC, EC, 2 * F], f8, tag="w1", name=f"w1_{l}", bufs=1)
                nc.sync.dma_start(out=w18, in_=r2(d_w18[l]))
                w28 = wp.tile([PC, 17, E], bf, tag="w2", name=f"w2_{l}", bufs=1)
                nc.sync.dma_start(out=w28, in_=d_w28[l].rearrange("(c p) o -> p c o", p=PC))
                if l != 0:
                    sm = wsm_next

                if l == 0:
                    # the mask is first read in the CA phase; it queues last
                    # so no layer-0 weight waits behind its 1.2MB
                    nc.sync.dma_start(out=maskb8, in_=d_maskb8)

                # ---- SA qkv projections (fp8 DR) ----
                q8_sa = acts.tile([PC, EC, NQ], f8, tag="q8sa", name=f"q8sa{l}")
                k8_sa = acts.tile([PC, EC, NQ], f8, tag="k8sa", name=f"k8sa{l}")
                pqa = ps_sc.tile([PC, 3, 512], f32, tag="sc", name=f"pqk{l}a")
                pqb = ps_sc.tile([PC, 3, 512], f32, tag="sc", name=f"pqk{l}b")
                for co in range(4):
                    po = (pqa, pqb)[co // 2][:, co % 2, 0:NQ]
                    nc.tensor.matmul(out=po,
                                     lhsT=wqk8[:, :, PC * co:PC * (co + 1)],
                                     rhs=tb8,
                                     start=True, stop=True, perf_mode=DR)
                for tt, (t0, tsz) in enumerate(TOK_TILES):
                    pv_t = ps_pp.tile([PC, 512], f32, tag="pp", name=f"pvsa{l}_{tt}")
                    for ci in range(EC):
                        nc.tensor.matmul(out=pv_t[0:tsz, 0:E],
                                         lhsT=tb8[:, ci, t0:t0 + tsz],
                                         rhs=wsv8[:, ci, :],
                                         start=(ci == 0), stop=(ci == EC - 1))
                    nc.vector.tensor_scalar_mul(
                        out=vsa[0:tsz, tt, :, 0:D],
                        in0=pv_t[0:tsz, 0:E].rearrange("p (h d) -> p h d", d=D),
                        scalar1=1.0 / WS)
                for co in range(4):
                    po = (pqa, pqb)[co // 2][:, co % 2, 0:NQ]
                    if co < 2:   # q: de-scale + permuted bias
                        nc.vector.tensor_scalar(
                            out=q8_sa[:, co, :], in0=po,
                            scalar1=1.0 / WS,
                            scalar2=sm[:, C_BQ_SA + co:C_BQ_SA + co + 1],
                            op0=Alu.mult, op1=Alu.add)
                    else:        # k: de-scale only (bias cancels in softmax)
                        nc.vector.tensor_scalar_mul(
                            out=k8_sa[:, co - 2, :], in0=po, scalar1=1.0 / WS)

                # ---- CA v-projection (fp8 DR, 2 key tiles per psum bank):
                # depends only on memT8/wcv8; WAR on vca (prev layer's CA
                # attention) is already clear. First pairs overlap the SA
                # attention phase. The ca v-bias is folded into the out-proj
                # bias host-side.
                def emit_vproj(lo, hi, l=l, wcv8=wcv8):
                    for kp in range(lo, hi):
                        pv2 = ps_pp.tile([PC, 512], f32, tag="pp", name=f"pvca{l}_{kp}")
                        pv2v = pv2.rearrange("p (j o) -> p j o", j=2)
                        for j in range(2):
                            kt = 2 * kp + j
                            nc.tensor.matmul(
                                out=pv2v[:, j, :],
                                lhsT=memT8[:, :, PC * kt:PC * (kt + 1)],
                                rhs=wcv8,
                                start=True, stop=True, perf_mode=DR)
                        nc.vector.tensor_scalar_mul(
                            out=vca[:, 2 * kp:2 * kp + 2, :, 0:D],
                            in0=pv2v.rearrange("p j (h d) -> p j h d", d=D),
                            scalar1=1.0 / WS)

                emit_vproj(0, 10)

                # ---- SA attention ----
                attn = sa_attention(q8_sa, k8_sa, f"sa{l}")

                # ---- SA out proj + LN1 (emits fp8 for CA q-proj) ----
                r = out_proj_residual(l, wso, attn, C_BO_SA, tT, f"so{l}")
                tT, tb8, _ = layernorm(l, r, C_LN, f"ln1_{l}")

                # ---- CA q projection (fp8 DR) -> fp8 head-packed q ----
                q8_ca = acts.tile([PC, EC, NQ], f8, tag="q_ca", name=f"q_ca{l}")
                pq = ps_sc.tile([PC, 3, 512], f32, tag="sc", name=f"pq{l}")
                for co in range(EC):
                    nc.tensor.matmul(
                        out=pq[:, co, 0:NQ],
                        lhsT=wcq8[:, :, PC * co:PC * (co + 1)],
                        rhs=tb8, start=True, stop=True, perf_mode=DR)
                for co in range(EC):
                    nc.vector.tensor_scalar(
                        out=q8_ca[:, co, :], in0=pq[:, co, 0:NQ],
                        scalar1=1.0 / WS,
                        scalar2=sm[:, C_BQ_CA + co:C_BQ_CA + co + 1],
                        op0=Alu.mult, op1=Alu.add)
                # ---- CA attention (k-projection fused into the scores) ----
                if l + 1 < nlayers:
                    sm_n = wp.tile([PC, NS], f32, tag="sm", name=f"sm{l + 1}")
                    nc.sync.dma_start(out=sm_n, in_=d_sm[l + 1])
                    wsm_next = sm_n
                attn = ca_attention(l, q8_ca, wckT8, f"ca{l}",
                                    bg_emit=lambda: emit_vproj(10, KP_CA),
                                    bg_at=10)

                # ---- CA out proj + LN2 ----
                r = out_proj_residual(l, wco, attn, C_BO_CA, tT, f"co{l}")
                tT, tb8, tb8r = layernorm(l, r, C_LN + 4, f"ln2_{l}", emit_resid=True)

                # ---- FFN: FFN1 fp8-DR with error-feedback (weight A|B
                # halves, input tb8+tb8r residual); hidden+FFN2 in bf16;
                # b2 folded in via the ones chunk ----
                hTb = acts.tile([PC, FT, NQ], bf, tag="hT", name=f"hT{l}", bufs=1)
                p2s = [ps_pp.tile([PC, 512], f32, tag="pp", name=f"pf2_{l}_{co}")
                       for co in range(EC)]
                done = [0]

                def emit_f2(upto):
                    # FFN2 accumulation steps for all hT chunks ready so far
                    while done[0] < upto:
                        fc = done[0]
                        for co in range(EC):
                            nc.tensor.matmul(out=p2s[co][:, 0:NQ],
                                             lhsT=w28[:, fc, PC * co:PC * (co + 1)],
                                             rhs=hTb[:, fc, :],
                                             start=(fc == 0), stop=False)
                        done[0] += 1

                for fg in range(0, FT, 3):
                    n = min(3, FT - fg)
                    pf = ps_sc.tile([PC, 3, 512], f32, tag="sc",
                                    name=f"pf1_{l}_{fg}")
                    for k in range(n):
                        ft = fg + k
                        nc.tensor.matmul(out=pf[:, k, 0:NQ],
                                         lhsT=w18[:, :, PC * ft:PC * (ft + 1)],
                                         rhs=tb8,
                                         start=True, stop=False, perf_mode=DR)
                        nc.tensor.matmul(out=pf[:, k, 0:NQ],
                                         lhsT=w18[:, :, PC * ft:PC * (ft + 1)],
                                         rhs=tb8r,
                                         start=False, stop=False, perf_mode=DR,
                                         skip_group_check=True)
                        nc.tensor.matmul(out=pf[:, k, 0:NQ],
                                         lhsT=w18[:, :, F + PC * ft:F + PC * (ft + 1)],
                                         rhs=tb8,
                                         start=False, stop=True, perf_mode=DR,
                                         skip_group_check=True)
                    emit_f2(max(0, fg - 2))  # FFN2 lags behind the relus
                    for k in range(n):
                        ft = fg + k
                        if ft % 2 == 0:
                            nc.scalar.activation(
                                out=hTb[:, ft, :], in_=pf[:, k, 0:NQ],
                                func=Act.Relu,
                                bias=sm[:, C_B1 + ft:C_B1 + ft + 1])
                        else:
                            nc.vector.tensor_scalar(
                                out=hTb[:, ft, :], in0=pf[:, k, 0:NQ],
                                scalar1=sm[:, C_B1 + ft:C_B1 + ft + 1], scalar2=0.0,
                                op0=Alu.add, op1=Alu.max)
                emit_f2(FT)
                # bias chunk: ones x (b2 row) closes the accumulation
                for co in range(EC):
                    nc.tensor.matmul(out=p2s[co][:, 0:NQ],
                                     lhsT=w28[:, 16, PC * co:PC * (co + 1)],
                                     rhs=honk,
                                     start=False, stop=True)
                r = acts.tile([PC, EC, NQ], f32, tag="r", name=f"rf{l}", bufs=1)
                for co in range(EC):
                    nc.vector.scalar_tensor_tensor(
                        out=r[:, co, :], in0=p2s[co][:, 0:NQ],
                        scalar=1.0 / WS,
                        in1=tT[:, co, :], op0=Alu.mult, op1=Alu.add)
                tT, tb8, _ = layernorm(l, r, C_LN + 8, f"ln3_{l}")

            # ---- final LN + store ----
            outT, _, _ = layernorm(None, tT, None, "lnf", emit=False)
            nc.sync.dma_start(out=r2(d_out), in_=outT)

    nc.compile()
    return nc


def _pack_inputs(inputs, nlayers=L):
    """Host-side layout prep: transpose / cast / permute / scale / pack."""
    smalls = np.zeros((nlayers, PC, NS), np.float32)
    for l in range(nlayers):
        def put(col, vec):
            n = vec.shape[0] // PC
            smalls[l, :, col:col + n] = vec.reshape(n, PC).T
        put(C_BQ_SA, np.asarray(inputs["sa_bqkv"][l][:E], np.float32))
        # v-biases folded into the out-proj biases (softmax rows sum to 1)
        sa_bv = np.asarray(inputs["sa_bqkv"][l][2 * E:], np.float32)
        bo_sa = np.asarray(inputs["sa_bo"][l], np.float32) + \
            np.asarray(inputs["sa_wo"][l], np.float32) @ sa_bv
        put(C_BO_SA, bo_sa)
        put(C_BQ_CA, np.asarray(inputs["ca_bq"][l], np.float32))
        bo_ca = np.asarray(inputs["ca_bo"][l], np.float32) + \
            np.asarray(inputs["ca_wo"][l], np.float32) @ \
            np.asarray(inputs["ca_bv"][l], np.float32)
        put(C_BO_CA, bo_ca)
        put(C_B1, np.asarray(inputs["f_b1"][l], np.float32) * WS)
        put(C_LN, np.asarray(inputs["ln1g"][l], np.float32))
        put(C_LN + 2, np.asarray(inputs["ln1b"][l], np.float32))
        put(C_LN + 4, np.asarray(inputs["ln2g"][l], np.float32))
        put(C_LN + 6, np.asarray(inputs["ln2b"][l], np.float32))
        put(C_LN + 8, np.asarray(inputs["ln3g"][l], np.float32))
        put(C_LN + 10, np.asarray(inputs["ln3b"][l], np.float32))
    finals = np.zeros((PC, 4), np.float32)
    finals[:, 0:2] = np.asarray(inputs["lnfg"], np.float32).reshape(2, PC).T
    finals[:, 2:4] = np.asarray(inputs["lnfb"], np.float32).reshape(2, PC).T

    def T(x):
        return np.ascontiguousarray(np.swapaxes(np.asarray(x), -1, -2))

    wqkv = np.asarray(inputs["sa_wqkv"], np.float32)[:nlayers]
    w_sa_qk = wqkv[:, :2 * E]
    w_sa_v = wqkv[:, 2 * E:]
    w_ca_q = np.asarray(inputs["ca_wq"], np.float32)[:nlayers]
    w_ca_k = np.asarray(inputs["ca_wk"], np.float32)[:nlayers]

    # FFN1 fp8 with error-feedback halves [L, E, 2F]
    w1s = T(np.asarray(inputs["f_w1"], np.float32)[:nlayers]) * WS  # [L, E, F]
    w1a = w1s.astype(FP8)
    w1b = (w1s - w1a.astype(np.float32)).astype(FP8)
    w_f18 = np.concatenate([w1a, w1b], axis=2)
    # FFN2 bf16 + bias chunk: [L, 17*128, E]
    w2 = T(np.asarray(inputs["f_w2"], np.float32)[:nlayers])        # [L, F, E]
    w2x = np.zeros((nlayers, PC, E), np.float32)
    w2x[:, 0, :] = np.asarray(inputs["f_b2"], np.float32)[:nlayers] * WS
    w_f28 = np.concatenate([w2, w2x], axis=1)

    # identity for the PE mask-bias accumulate: key m = 64c + p
    # value 16: carries the x16 qw scale into the mask-bias accumulate
    ident = np.zeros((64, 2, PC), np.float32)
    for p in range(64):
        for c in range(2):
            ident[p, c, 64 * c + p] = 16.0

    shared = {
        "w_sa_qk8": (T(w_sa_qk) * WS).astype(FP8),
        "w_sa_v8": (T(w_sa_v) * WS).astype(FP8),
        "w_sa_o": T(inputs["sa_wo"][:nlayers]).astype(BF16),
        "w_ca_q8": (T(w_ca_q) * WS).astype(FP8),
        "w_ca_k8": (w_ca_k * WS).astype(FP8),
        "w_ca_v8": (T(np.asarray(inputs["ca_wv"], np.float32)[:nlayers]) * WS).astype(FP8),
        "w_ca_o": T(inputs["ca_wo"][:nlayers]).astype(BF16),
        "w_f18": w_f18,
        "w_f28": w_f28.astype(BF16),
        "smalls": smalls,
        "finals": finals,
        "ident8": ident.astype(FP8),
    }
    in_maps = []
    for b in range(B):
        m = dict(shared)
        m["tT"] = T(inputs["tgt"][b]).astype(np.float32)
        m["memT8"] = T(inputs["memory"][b]).astype(FP8)
        # mask bias [64, KT, 2, NQ]: key k = 128*kt + 64*c + p
        mk = np.asarray(inputs["geometry_mask"][b])       # [NQ, NK] bool
        mb = np.where(mk.T, 0.0, MASKB).astype(np.float32)  # [NK, NQ]
        m["maskb8"] = np.ascontiguousarray(
            mb.reshape(KT_CA, 2, 64, NQ).transpose(2, 0, 1, 3)).astype(FP8)
        in_maps.append(m)
    return in_maps


_CACHE = {}


def kernel(run_opts=None, **inputs):
    nlayers = L
    if "nc" not in _CACHE:
        _CACHE["nc"] = build_nc(nlayers)
    nc = _CACHE["nc"]
    in_maps = _pack_inputs(inputs, nlayers)
    res = bass_utils.run_bass_kernel_spmd(
        nc, in_maps, core_ids=list(range(B)), **(run_opts or {}))
    _CACHE["last_result"] = res
    out = np.stack([np.asarray(r["outT"]).T for r in res.results])
    return np.ascontiguousarray(out.astype(np.float32))


# revision 18
# speedup vs baseline: 1.0227x; 1.0227x over previous
"""Trainium2 Bass kernel for a 6-layer geometry-constrained cross-attention decoder.

Sharding: pure data-parallel over batch B=8 -> one batch element per NeuronCore.
Per-core layouts are feature-major ("T" = transposed): activations live as
[feature, token].

Fully fp8-DoubleRow matmul pipeline (0.5 PE-cycles per output row):
- CA/SA attention q/k/scores and probabilities are fp8 end to end.
- The geometry mask is applied on the PE: an fp8 identity matmul accumulates
  a {0, -176} mask bias into the scores PSUM ahead of the exp, so the former
  per-group DVE mask multiply disappears entirely.
- Softmax exp emits fp8 probabilities straight from the Act engine (free);
  AV contracts 256 keys per DR pass against fp8 V (ones rows in the V tile
  produce the softmax denominator in the same pass).
- FFN runs fp8-DR end to end; weights are scaled x32 into e4m3's normal
  range and de-scaled inside the bias/relu stages. The FFN2 output bias is
  folded in as an extra contraction pair against a persistent ones vector.
- LayerNorm rstd = exp(-0.5*ln(var+eps)); ln/exp share one activation table
  (compile-time table hint) so the Act engine never reloads tables.
- The next layer's k-projection is interleaved into the CA attention heads'
  PE slack; the v-projection overlaps the SA attention phase.

Residual stream, layernorm statistics, biases and PSUM accumulation in fp32.
"""

import os
import sys

for _p in ("/opt/trn_rl_repo", "/root/.axon_site/_ro/trn_rl_repo"):
    if os.path.isdir(_p) and _p not in sys.path:
        sys.path.insert(0, _p)

import numpy as np
import ml_dtypes

import concourse.bass as bass
import concourse.tile as tile
from concourse import bacc
from concourse import mybir
from concourse import bass_utils

BF16 = ml_dtypes.bfloat16
FP8 = ml_dtypes.float8_e4m3
F32 = np.float32

B, NQ, NK, E, H, F, L = 8, 300, 4096, 256, 8, 2048, 6
D = E // H
SCALE = D ** -0.5
PC = 128          # partitions
EC = E // PC      # 2 feature chunks
FT = F // PC      # 16 ffn chunks
KT_CA = NK // PC  # 32 cross-attention key tiles
KP_CA = KT_CA // 2  # 16 DR key-tile pairs
TOK_TILES = [(0, 100), (100, 100), (200, 100)]   # 300 tokens, uniform
WS = 32.0         # fp8 weight scaling (into e4m3 normal range)
MASKB = -176.0    # additive mask bias (exp(SCALE*-176) ~ 5e-14)
A_SCH = 2.0 ** 23 / np.log(2.0)   # Schraudolph exp slope
B_SCH = 1065353216.0              # 127 * 2^23 (bf16-exact)
DVE_HEADS = (1, 3, 5)             # CA heads whose exp runs on the DVE

dt = mybir.dt
Alu = mybir.AluOpType
Act = mybir.ActivationFunctionType
DR = mybir.MatmulPerfMode.DoubleRow

# smalls column map (per-partition fp32 vectors, feature f = 128*c + p)
C_BQ_SA = 0   # 2 cols: sa q bias
C_BO_SA = 4   # 2 (includes folded sa v-bias)
C_BQ_CA = 6   # 2
C_BO_CA = 8   # 2 (includes folded ca v-bias)
C_B1 = 12     # 16 (x WS)
C_LN = 30     # 12: ln1g ln1b ln2g ln2b ln3g ln3b (2 each)
NS = 42


def _bcmid(ap2d, c):
    """[P, N] AP -> [P, c, N] with the middle dim broadcast (step 0)."""
    return bass.AP(tensor=ap2d.tensor, offset=ap2d.offset,
                   ap=[list(ap2d.ap[0]), [0, c], list(ap2d.ap[-1])])


def _patch_act_tables():
    """Compile-time hint: make Exp/Ln resolve to the one table set that
    contains both ('natural_log_exp_and_others'), so the greedy table-load
    pass emits a single load instead of thrashing between sets. Set ids and
    contents seen by the NEFF compiler are unchanged."""
    from concourse import hw_specs as _hw
    from concourse import bacc as _bacc
    if getattr(_hw, "_act_tables_patched", False):
        return
    orig = _hw.get_activation_tables

    def patched(arch):
        t = dict(orig(arch))
        A = mybir.ActivationFunctionType
        keep = "natural_log_exp_and_others"
        if keep in t and A.Exp in t[keep] and A.Ln in t[keep]:
            t = {name: (funcs if name == keep else funcs - {A.Exp, A.Ln})
                 for name, funcs in t.items()}
        return t

    _hw.get_activation_tables = patched
    _hw._act_tables_patched = True
    if getattr(_bacc, "get_activation_tables", None) is orig:
        _bacc.get_activation_tables = patched


def build_nc(nlayers=L):
    _patch_act_tables()
    nc = bacc.Bacc("TRN2", target_bir_lowering=False, debug=False)
    f32, bf, f8 = dt.float32, dt.bfloat16, dt.float8e4

    def din(name, shape, d=bf):
        return nc.dram_tensor(name, shape, d, kind="ExternalInput").ap()

    d_tT = din("tT", [E, NQ], f32)
    d_memT8 = din("memT8", [E, NK], f8)
    d_maskb8 = din("maskb8", [64, KT_CA, 2, NQ], f8)
    d_maskbs = din("maskbs", [PC, KT_CA, NQ])
    d_ident8 = din("ident8", [64, 2, PC], f8)
    d_wqk8 = din("w_sa_qk8", [nlayers, E, 2 * E], f8)
    d_wsv8 = din("w_sa_v8", [nlayers, E, E], f8)
    d_wso = din("w_sa_o", [nlayers, E, E])
    d_wcq8 = din("w_ca_q8", [nlayers, E, E], f8)
    d_wck8 = din("w_ca_k8", [nlayers, E, E], f8)
    d_wcv8 = din("w_ca_v8", [nlayers, E, E], f8)
    d_wco = din("w_ca_o", [nlayers, E, E])
    d_w18 = din("w_f18", [nlayers, E, 2 * F], f8)
    d_w28 = din("w_f28", [nlayers, 17 * PC, E])
    d_sm = din("smalls", [nlayers, PC, NS], f32)
    d_fin = din("finals", [PC, 4], f32)
    d_out = nc.dram_tensor("outT", [E, NQ], f32, kind="ExternalOutput").ap()

    def r2(ap):  # [256, X] -> [128, 2, X]
        return ap.rearrange("(c p) o -> p c o", p=PC)

    with tile.TileContext(nc) as tc:
        with (
            tc.tile_pool(name="persist", bufs=1) as pst,
            tc.tile_pool(name="wts", bufs=2) as wp,
            tc.tile_pool(name="acts", bufs=2) as acts,
            tc.tile_pool(name="probs", bufs=6) as probs,
            tc.tile_pool(name="stats", bufs=2) as stp,
            tc.tile_pool(name="ps_sc", bufs=2, space="PSUM") as ps_sc,
            tc.tile_pool(name="ps_pp", bufs=2, space="PSUM") as ps_pp,
        ):
            # ---- persistent loads (memT8/maskb8 queued after tT: they are
            # only needed from the CA phase on, tT feeds layer 0's SA) ----
            memT8 = pst.tile([PC, EC, NK], f8, tag="memT8", name="memT8_sb")
            maskb8 = pst.tile([64, KT_CA, 2, NQ], f8, tag="maskb8", name="maskb8_sb")
            maskbs = pst.tile([PC, KT_CA, NQ], bf, tag="maskbs", name="maskbs_sb")
            ident8 = pst.tile([64, 2, PC], f8, tag="ident8", name="ident8_sb")
            eps = pst.tile([PC, 1], f32, tag="eps", name="eps_sb")
            nc.vector.memset(eps, 1e-5)
            ones = pst.tile([PC, PC], bf, tag="ones", name="ones_sb")
            nc.vector.memset(ones, 1.0)
            fin = pst.tile([PC, 4], f32, tag="fin", name="fin_sb")
            nc.sync.dma_start(out=fin, in_=d_fin)
            honk = pst.tile([PC, NQ], bf, tag="honk", name="honk_sb")
            nc.gpsimd.memset(honk, 1.0)
            vsa = pst.tile([PC, 3, H, 2 * D], f8, tag="vsa", name="vsa_sb")
            nc.gpsimd.memset(vsa[:, :, :, D:2 * D], 1.0)
            vca = pst.tile([PC, KT_CA, H, 2 * D], f8, tag="vca", name="vca_sb")
            nc.gpsimd.memset(vca[:, :, :, D:2 * D], 1.0)

            tT = acts.tile([PC, EC, NQ], f32, tag="tT", name="tT0")
            nc.sync.dma_start(out=tT, in_=r2(d_tT))
            tb8 = acts.tile([PC, EC, NQ], f8, tag="tb8", name="tb8_0")
            nc.gpsimd.tensor_copy(out=tb8, in_=tT)
            nc.sync.dma_start(out=memT8, in_=r2(d_memT8))
            nc.sync.dma_start(out=ident8, in_=d_ident8)

            def layernorm(l, r, gcol, name, emit=True, emit_resid=False):
                """r: [128, 2, 300] f32 -> (t_new f32, tb8_new fp8-or-None)"""
                rb = acts.tile([PC, EC, NQ], bf, tag="rb", name=f"rb{name}", bufs=1)
                nc.vector.tensor_copy(out=rb, in_=r)
                tsq = acts.tile([PC, EC, NQ], bf, tag="tsq", name=f"tsq{name}", bufs=1)
                nc.vector.tensor_mul(out=tsq, in0=rb, in1=rb)
                s0 = ps_pp.tile([PC, 512], f32, tag="pp", name=f"lns0{name}")
                s1 = ps_pp.tile([PC, 512], f32, tag="pp", name=f"lns1{name}")
                for c in range(EC):
                    nc.tensor.matmul(out=s0[:, 0:NQ], lhsT=ones,
                                     rhs=rb[:, c, :],
                                     start=(c == 0), stop=(c == EC - 1))
                for c in range(EC):
                    nc.tensor.matmul(out=s1[:, 0:NQ], lhsT=ones,
                                     rhs=tsq[:, c, :],
                                     start=(c == 0), stop=(c == EC - 1))
                # stats chain stays on one engine (DVE) in dependency order so
                # the Act ln/exp can start as early as possible; c1 follows.
                mean = stp.tile([PC, NQ], f32, tag="mean", name=f"mean{name}", bufs=1)
                nc.vector.tensor_scalar_mul(out=mean, in0=s0[:, 0:NQ], scalar1=1.0 / E)
                msq = stp.tile([PC, NQ], f32, tag="msq", name=f"msq{name}", bufs=1)
                nc.vector.tensor_mul(out=msq, in0=mean, in1=mean)
                var = stp.tile([PC, NQ], f32, tag="var", name=f"var{name}", bufs=1)
                nc.vector.scalar_tensor_tensor(out=var, in0=s1[:, 0:NQ], scalar=1.0 / E,
                                               in1=msq, op0=Alu.mult, op1=Alu.subtract)
                # rstd = (var + eps)^-0.5 via ln/exp (same act table as Exp)
                lnv = stp.tile([PC, NQ], f32, tag="lnv", name=f"lnv{name}", bufs=1)
                nc.scalar.activation(out=lnv, in_=var, func=Act.Ln, bias=eps[:, 0:1])
                rstd = stp.tile([PC, NQ], f32, tag="rstd", name=f"rstd{name}", bufs=1)
                nc.scalar.activation(out=rstd, in_=lnv, func=Act.Exp, scale=-0.5)
                c1 = acts.tile([PC, EC, NQ], f32, tag="c1", name=f"c1{name}", bufs=1)
                nc.vector.tensor_sub(out=c1, in0=r, in1=_bcmid(mean, EC))
                c2 = acts.tile([PC, EC, NQ], f32, tag="c2", name=f"c2{name}", bufs=1)
                nc.vector.tensor_mul(out=c2, in0=c1, in1=_bcmid(rstd, EC))
                t_new = acts.tile([PC, EC, NQ], f32, tag="tT", name=f"t{name}")
                if gcol is None:
                    g, b = fin[:, 0:2], fin[:, 2:4]
                else:
                    g = sm[:, gcol:gcol + 2]
                    b = sm[:, gcol + 2:gcol + 4]
                tb8_new = None
                if emit:
                    tb8_new = acts.tile([PC, EC, NQ], f8, tag="tb8", name=f"tb{name}")
                for c in range(EC):
                    if emit:
                        nc.vector.tensor_scalar(out=tb8_new[:, c, :], in0=c2[:, c, :],
                                                scalar1=g[:, c:c + 1], scalar2=b[:, c:c + 1],
                                                op0=Alu.mult, op1=Alu.add)
                    nc.gpsimd.tensor_scalar(out=t_new[:, c, :], in0=c2[:, c, :],
                                            scalar1=g[:, c:c + 1], scalar2=b[:, c:c + 1],
                                            op0=Alu.mult, op1=Alu.add)
                tb8_res = None
                if emit_resid:
                    # fp8 error-feedback residual of the emit (for FFN1)
                    tb8_res = acts.tile([PC, EC, NQ], f8, tag="tb8r", name=f"tbr{name}")
                    nc.vector.scalar_tensor_tensor(
                        out=tb8_res, in0=tb8_new, scalar=-1.0, in1=t_new,
                        op0=Alu.mult, op1=Alu.add)
                return t_new, tb8_new, tb8_res

            def sa_attention(q8, k8, name, per_head_emit=None):
                """SA fp8 attention. q8/k8 [128, 2, 300] feature-major; vsa
                [128(100), 3, H, 64] fp8; returns attn [128, 2, 300] bf16.
                Scores for head h+1 are emitted before head h's AV so the
                exps run back-to-back."""
                attn = acts.tile([PC, EC, NQ], bf, tag="attn", name=f"attn{name}")
                nkt = len(TOK_TILES)

                def emit_sc(h):
                    po = 32 * (h % 4)
                    ci = h // 4
                    sc = ps_sc.tile([PC, 3, 512], f32, tag="sc", name=f"sc{name}h{h}")
                    for j in range(nkt):
                        kt0, ksz = TOK_TILES[j]
                        nc.tensor.matmul(
                            out=sc[0:ksz, j, 0:NQ],
                            lhsT=k8[po:po + 32, ci, kt0:kt0 + ksz],
                            rhs=q8[po:po + 32, ci, 0:NQ],
                            start=True, stop=True,
                            tile_position=(po, 0))
                    return sc

                sc = emit_sc(0)
                for h in range(H):
                    po = 32 * (h % 4)
                    ci = h // 4
                    av = ps_pp.tile([PC, 512], f32, tag="pp", name=f"av{name}h{h}")
                    p8 = probs.tile([PC, 3, NQ], f8, tag="p",
                                    name=f"p{name}h{h}", bufs=10)
                    nc.scalar.activation(out=p8[0:100, 0:3, :],
                                         in_=sc[0:100, 0:3, 0:NQ], func=Act.Exp,
                                         scale=SCALE)
                    if h + 1 < H:
                        sc = emit_sc(h + 1)
                    if per_head_emit is not None:
                        per_head_emit(h)
                    nc.tensor.matmul(
                        out=av[0:2 * D, 0:NQ],
                        lhsT=vsa[0:100, 0:2, h, 0:2 * D],
                        rhs=p8[0:100, 0:2, 0:NQ],
                        start=True, stop=False, perf_mode=DR,
                        skip_group_check=True)
                    nc.tensor.matmul(
                        out=av[0:2 * D, 0:NQ],
                        lhsT=vsa[0:100, 2, h, 0:2 * D],
                        rhs=p8[0:100, 2, 0:NQ],
                        start=False, stop=True,
                        skip_group_check=True)
                    recip = stp.tile([32, NQ], f32, tag="recip",
                                     name=f"rc{name}h{h}", bufs=4)
                    nc.vector.reciprocal(out=recip, in_=av[D:2 * D, 0:NQ])
                    nc.vector.tensor_mul(out=attn[po:po + 32, ci, :],
                                         in0=av[0:D, 0:NQ], in1=recip)
                return attn

            def ca_attention(l, q8, wckT8, name, bg_emit=None, bg_at=10,
                             per_head_emit=None):
                """k-projection fused into the scores: per head,
                qw_h = fp8(16 * Wk_h^T q_h); then 16*s_h = memT8^T qw_h as an
                fp8-DR matmul over the 256 memory features (the k-tensor is
                never materialized). The identity mask-add carries value 16 so
                the {0,-176} bias lands in the 16x-scaled PSUM; exp applies
                SCALE/16. fp8-DR AV with ones-rows for the denominator."""
                attn = acts.tile([PC, EC, NQ], bf, tag="attn", name=f"attn{name}")
                qws = {}

                def emit_qw(h):
                    po = 32 * (h % 4)
                    ci = h // 4
                    qwp = ps_sc.tile([PC, 3, 512], f32, tag="sc",
                                     name=f"qwp{name}h{h}")
                    for ec in range(EC):
                        nc.tensor.matmul(
                            out=qwp[:, ec, 0:NQ],
                            lhsT=wckT8[po:po + 32, ci, PC * ec:PC * (ec + 1)],
                            rhs=q8[po:po + 32, ci, 0:NQ],
                            start=True, stop=True,
                            tile_position=(po, 0))
                    qw8 = probs.tile([PC, EC, NQ], f8, tag="qw",
                                     name=f"qw{name}h{h}", bufs=4)
                    nc.vector.tensor_scalar_mul(out=qw8, in0=qwp[:, 0:EC, 0:NQ],
                                                scalar1=0.5)
                    qws[h] = qw8
                groups = []
                g = 0
                while g < KT_CA:
                    groups.append((g, min(3, KT_CA - g)))
                    g += groups[-1][1]
                NG = len(groups)
                # DVE-exp heads are interleaved group-wise with an Act-exp
                # partner so both engines run softmax concurrently
                order, tasks = [], []
                rest = [h for h in range(H) if h not in DVE_HEADS]
                for i, b in enumerate(DVE_HEADS):
                    a = rest[i]
                    order += [a, b]
                    for gi in range(NG):
                        tasks += [(a, gi), (b, gi)]
                for h in rest[len(DVE_HEADS):]:
                    order.append(h)
                    tasks += [(h, gi) for gi in range(NG)]
                nxt = {h: order[i + 2] for i, h in enumerate(order)
                       if i + 2 < len(order)}
                avs = {}

                def emit_sc(h, gi):
                    g0, gsz = groups[gi]
                    sc = ps_sc.tile([PC, 3, 512], f32, tag="sc",
                                    name=f"sc{name}h{h}g{g0}")
                    masked = h not in DVE_HEADS
                    for j in range(gsz):
                        kt = g0 + j
                        nc.tensor.matmul(
                            out=sc[0:PC, j, 0:NQ],
                            lhsT=memT8[:, :, PC * kt:PC * (kt + 1)],
                            rhs=qws[h],
                            start=True, stop=not masked, perf_mode=DR,
                            skip_group_check=True)
                        if masked:
                            nc.tensor.matmul(
                                out=sc[0:PC, j, 0:NQ],
                                lhsT=ident8,
                                rhs=maskb8[:, kt, :, :],
                                start=False, stop=True, perf_mode=DR,
                                skip_group_check=True)
                    return sc

                def emit_av(h, g0, gsz, p8):
                    if h in DVE_HEADS:
                        # bf16 high-half probs: per-tile non-DR passes
                        for j in range(gsz):
                            nc.tensor.matmul(
                                out=avs[h][0:2 * D, 0:NQ],
                                lhsT=vca[:, g0 + j, h, 0:2 * D],
                                rhs=p8[:, j, 0:NQ],
                                start=(g0 + j == 0), stop=(g0 + j == KT_CA - 1),
                                skip_group_check=True)
                        return
                    # DR over the leading pair, single pass for the tail tile
                    if gsz >= 2:
                        nc.tensor.matmul(
                            out=avs[h][0:2 * D, 0:NQ],
                            lhsT=vca[:, g0:g0 + 2, h, 0:2 * D],
                            rhs=p8[:, 0:2, 0:NQ],
                            start=(g0 == 0), stop=(g0 + gsz == KT_CA and gsz == 2),
                            perf_mode=DR, skip_group_check=True)
                    if gsz != 2:
                        j = gsz - 1
                        nc.tensor.matmul(
                            out=avs[h][0:2 * D, 0:NQ],
                            lhsT=vca[:, g0 + j, h, 0:2 * D],
                            rhs=p8[:, j, 0:NQ],
                            start=(g0 == 0 and gsz == 1), stop=(g0 + gsz == KT_CA),
                            skip_group_check=True)

                def finish_head(h):
                    po = 32 * (h % 4)
                    ci = h // 4
                    recip = stp.tile([32, NQ], f32, tag="recip",
                                     name=f"rc{name}h{h}", bufs=4)
                    nc.vector.reciprocal(out=recip, in_=avs[h][D:2 * D, 0:NQ])
                    nc.vector.tensor_mul(out=attn[po:po + 32, ci, :],
                                         in0=avs[h][0:D, 0:NQ], in1=recip)

                # flat (head, pair) pipeline: exp for task i, scores for task
                # i+1, then the (lagged) AV of task i-1 — so neither a head
                # boundary nor the exp ever head-of-line blocks the streams.
                emit_qw(order[0])
                emit_qw(order[1])
                sc = emit_sc(order[0], 0)
                pend = None
                for idx, (h, gi) in enumerate(tasks):
                    g0, gsz = groups[gi]
                    if gi == 0:
                        avs[h] = ps_pp.tile([PC, 512], f32, tag="pp",
                                            name=f"av{name}h{h}")
                    if h in DVE_HEADS:
                        pI = probs.tile([PC, 3, NQ], dt.int32, tag="pI",
                                        name=f"pI{name}h{h}g{g0}", bufs=4)
                        nc.vector.scalar_tensor_tensor(
                            out=pI[:, 0:gsz, :], in0=sc[:, 0:gsz, 0:NQ],
                            scalar=A_SCH * SCALE / 16.0,
                            in1=maskbs[:, g0:g0 + gsz, :],
                            op0=Alu.mult, op1=Alu.add)
                        p8 = pI.bitcast(bf)[:, :, 1::2]
                    else:
                        p8 = probs.tile([PC, 3, NQ], f8, tag="p",
                                        name=f"p{name}h{h}g{g0}", bufs=10)
                        nc.scalar.activation(out=p8[:, 0:gsz, :],
                                             in_=sc[:, 0:gsz, 0:NQ], func=Act.Exp,
                                             scale=SCALE / 16.0)
                    if idx + 1 < len(tasks):
                        sc = emit_sc(*tasks[idx + 1])
                    # the rest of the v-projection must be in the PE stream
                    # before any AV matmul that reads vca[2*bg_at:]
                    if h == 0 and bg_emit is not None and g0 + gsz > 2 * bg_at - 3:
                        bg_emit()
                        bg_emit = None
                    if pend is not None:
                        ph, pg0, pgsz, pp8 = pend
                        emit_av(ph, pg0, pgsz, pp8)
                        if pg0 + pgsz == KT_CA:
                            finish_head(ph)
                    if gi == 5:
                        if h in nxt:
                            emit_qw(nxt[h])
                        if per_head_emit is not None:
                            per_head_emit(h)   # mid-head: away from the boundary
                    pend = (h, g0, gsz, p8)
                ph, pg0, pgsz, pp8 = pend
                emit_av(ph, pg0, pgsz, pp8)
                finish_head(ph)
                return attn

            def out_proj_residual(l, w_sb, attn, bcol, tT, name):
                r = acts.tile([PC, EC, NQ], f32, tag="r", name=f"r{name}", bufs=1)
                pos = ps_sc.tile([PC, 3, 512], f32, tag="sc", name=f"po{name}")
                for co in range(EC):
                    for ci in range(EC):
                        nc.tensor.matmul(out=pos[:, co, 0:NQ],
                                         lhsT=w_sb[:, ci, PC * co:PC * (co + 1)],
                                         rhs=attn[:, ci, :],
                                         start=(ci == 0), stop=(ci == EC - 1))
                for co in range(EC):
                    nc.vector.scalar_tensor_tensor(
                        out=r[:, co, :], in0=pos[:, co, 0:NQ],
                        scalar=sm[:, bcol + co:bcol + co + 1],
                        in1=tT[:, co, :], op0=Alu.add, op1=Alu.add)
                return r

            wsm_next = None
            for l in range(nlayers):
                # ---- layer weight loads (smalls first: the first SA
                # bias op waits on it) ----
                if l == 0:
                    sm = wp.tile([PC, NS], f32, tag="sm", name=f"sm{l}")
                    nc.sync.dma_start(out=sm, in_=d_sm[l])
                wqk8 = wp.tile([PC, EC, 2 * E], f8, tag="wqk", name=f"wqk{l}")
                nc.sync.dma_start(out=wqk8, in_=r2(d_wqk8[l]))
                wsv8 = wp.tile([PC, EC, E], f8, tag="wsv", name=f"wsv{l}")
                nc.sync.dma_start(out=wsv8, in_=r2(d_wsv8[l]))
                wso = wp.tile([PC, EC, E], bf, tag="wso", name=f"wso{l}")
                nc.sync.dma_start(out=wso, in_=r2(d_wso[l]))

                wcq8 = wp.tile([PC, EC, E], f8, tag="wcq8", name=f"wcq8{l}")
                nc.sync.dma_start(out=wcq8, in_=r2(d_wcq8[l]))
                wckT8 = wp.tile([PC, EC, E], f8, tag="wck8", name=f"wck8{l}")
                nc.sync.dma_start(out=wckT8, in_=r2(d_wck8[l]))
                wcv8 = wp.tile([PC, EC, E], f8, tag="wcv8", name=f"wcv8{l}")
                nc.sync.dma_start(out=wcv8, in_=r2(d_wcv8[l]))
                wco = wp.tile([PC, EC, E], bf, tag="wco", name=f"wco{l}")
                nc.sync.dma_start(out=wco, in_=r2(d_wco[l]))
                w18 = wp.tile([PC, EC, 2 * F], f8, tag="w1", name=f"w1_{l}", bufs=1)
                nc.sync.dma_start(out=w18, in_=r2(d_w18[l]))
                w28 = wp.tile([PC, 17, E], bf, tag="w2", name=f"w2_{l}", bufs=1)
                nc.sync.dma_start(out=w28, in_=d_w28[l].rearrange("(c p) o -> p c o", p=PC))
                if l != 0:
                    sm = wsm_next

                if l == 0:
                    # masks are first read in the CA phase; they queue last
                    # so no layer-0 weight waits behind their 3.6MB
                    nc.sync.dma_start(out=maskb8, in_=d_maskb8)
                    nc.sync.dma_start(out=maskbs, in_=d_maskbs)

                # ---- SA qkv projections (fp8 DR) ----
                q8_sa = acts.tile([PC, EC, NQ], f8, tag="q8sa", name=f"q8sa{l}")
                k8_sa = acts.tile([PC, EC, NQ], f8, tag="k8sa", name=f"k8sa{l}")
                pqa = ps_sc.tile([PC, 3, 512], f32, tag="sc", name=f"pqk{l}a")
                pqb = ps_sc.tile([PC, 3, 512], f32, tag="sc", name=f"pqk{l}b")
                for co in range(4):
                    po = (pqa, pqb)[co // 2][:, co % 2, 0:NQ]
                    nc.tensor.matmul(out=po,
                                     lhsT=wqk8[:, :, PC * co:PC * (co + 1)],
                                     rhs=tb8,
                                     start=True, stop=True, perf_mode=DR)
                for tt, (t0, tsz) in enumerate(TOK_TILES):
                    pv_t = ps_pp.tile([PC, 512], f32, tag="pp", name=f"pvsa{l}_{tt}")
                    for ci in range(EC):
                        nc.tensor.matmul(out=pv_t[0:tsz, 0:E],
                                         lhsT=tb8[:, ci, t0:t0 + tsz],
                                         rhs=wsv8[:, ci, :],
                                         start=(ci == 0), stop=(ci == EC - 1))
                    nc.vector.tensor_scalar_mul(
                        out=vsa[0:tsz, tt, :, 0:D],
                        in0=pv_t[0:tsz, 0:E].rearrange("p (h d) -> p h d", d=D),
                        scalar1=1.0 / WS)
                for co in range(4):
                    po = (pqa, pqb)[co // 2][:, co % 2, 0:NQ]
                    if co < 2:   # q: de-scale + permuted bias
                        nc.vector.tensor_scalar(
                            out=q8_sa[:, co, :], in0=po,
                            scalar1=1.0 / WS,
                            scalar2=sm[:, C_BQ_SA + co:C_BQ_SA + co + 1],
                            op0=Alu.mult, op1=Alu.add)
                    else:        # k: de-scale only (bias cancels in softmax)
                        nc.vector.tensor_scalar_mul(
                            out=k8_sa[:, co - 2, :], in0=po, scalar1=1.0 / WS)

                # ---- CA v-projection (fp8 DR, 2 key tiles per psum bank):
                # depends only on memT8/wcv8; WAR on vca (prev layer's CA
                # attention) is already clear. First pairs overlap the SA
                # attention phase. The ca v-bias is folded into the out-proj
                # bias host-side.
                def emit_vproj(lo, hi, l=l, wcv8=wcv8):
                    for kp in range(lo, hi):
                        pv2 = ps_pp.tile([PC, 512], f32, tag="pp", name=f"pvca{l}_{kp}")
                        pv2v = pv2.rearrange("p (j o) -> p j o", j=2)
                        for j in range(2):
                            kt = 2 * kp + j
                            nc.tensor.matmul(
                                out=pv2v[:, j, :],
                                lhsT=memT8[:, :, PC * kt:PC * (kt + 1)],
                                rhs=wcv8,
                                start=True, stop=True, perf_mode=DR)
                        nc.vector.tensor_scalar_mul(
                            out=vca[:, 2 * kp:2 * kp + 2, :, 0:D],
                            in0=pv2v.rearrange("p j (h d) -> p j h d", d=D),
                            scalar1=1.0 / WS)

                emit_vproj(0, KP_CA)

                # ---- SA attention ----
                attn = sa_attention(q8_sa, k8_sa, f"sa{l}")

                # ---- SA out proj + LN1 (emits fp8 for CA q-proj) ----
                r = out_proj_residual(l, wso, attn, C_BO_SA, tT, f"so{l}")
                tT, tb8, _ = layernorm(l, r, C_LN, f"ln1_{l}")

                # ---- CA q projection (fp8 DR) -> fp8 head-packed q ----
                q8_ca = acts.tile([PC, EC, NQ], f8, tag="q_ca", name=f"q_ca{l}")
                pq = ps_sc.tile([PC, 3, 512], f32, tag="sc", name=f"pq{l}")
                for co in range(EC):
                    nc.tensor.matmul(
                        out=pq[:, co, 0:NQ],
                        lhsT=wcq8[:, :, PC * co:PC * (co + 1)],
                        rhs=tb8, start=True, stop=True, perf_mode=DR)
                for co in range(EC):
                    nc.vector.tensor_scalar(
                        out=q8_ca[:, co, :], in0=pq[:, co, 0:NQ],
                        scalar1=1.0 / WS,
                        scalar2=sm[:, C_BQ_CA + co:C_BQ_CA + co + 1],
                        op0=Alu.mult, op1=Alu.add)
                # ---- CA attention (k-projection fused into the scores) ----
                if l + 1 < nlayers:
                    sm_n = wp.tile([PC, NS], f32, tag="sm", name=f"sm{l + 1}")
                    nc.sync.dma_start(out=sm_n, in_=d_sm[l + 1])
                    wsm_next = sm_n
                attn = ca_attention(l, q8_ca, wckT8, f"ca{l}")

                # ---- CA out proj + LN2 ----
                r = out_proj_residual(l, wco, attn, C_BO_CA, tT, f"co{l}")
                tT, tb8, tb8r = layernorm(l, r, C_LN + 4, f"ln2_{l}", emit_resid=True)

                # ---- FFN: FFN1 fp8-DR with error-feedback (weight A|B
                # halves, input tb8+tb8r residual); hidden+FFN2 in bf16;
                # b2 folded in via the ones chunk ----
                hTb = acts.tile([PC, FT, NQ], bf, tag="hT", name=f"hT{l}", bufs=1)
                p2s = [ps_pp.tile([PC, 512], f32, tag="pp", name=f"pf2_{l}_{co}")
                       for co in range(EC)]
                done = [0]

                def emit_f2(upto):
                    # FFN2 accumulation steps for all hT chunks ready so far
                    while done[0] < upto:
                        fc = done[0]
                        for co in range(EC):
                            nc.tensor.matmul(out=p2s[co][:, 0:NQ],
                                             lhsT=w28[:, fc, PC * co:PC * (co + 1)],
                                             rhs=hTb[:, fc, :],
                                             start=(fc == 0), stop=False)
                        done[0] += 1

                for fg in range(0, FT, 3):
                    n = min(3, FT - fg)
                    pf = ps_sc.tile([PC, 3, 512], f32, tag="sc",
                                    name=f"pf1_{l}_{fg}")
                    for k in range(n):
                        ft = fg + k
                        nc.tensor.matmul(out=pf[:, k, 0:NQ],
                                         lhsT=w18[:, :, PC * ft:PC * (ft + 1)],
                                         rhs=tb8,
                                         start=True, stop=False, perf_mode=DR)
                        nc.tensor.matmul(out=pf[:, k, 0:NQ],
                                         lhsT=w18[:, :, PC * ft:PC * (ft + 1)],
                                         rhs=tb8r,
                                         start=False, stop=False, perf_mode=DR,
                                         skip_group_check=True)
                        nc.tensor.matmul(out=pf[:, k, 0:NQ],
                                         lhsT=w18[:, :, F + PC * ft:F + PC * (ft + 1)],
                                         rhs=tb8,
                                         start=False, stop=True, perf_mode=DR,
                                         skip_group_check=True)
                    emit_f2(max(0, fg - 2))  # FFN2 lags behind the relus
                    for k in range(n):
                        ft = fg + k
                        if ft % 2 == 0:
                            nc.scalar.activation(
                                out=hTb[:, ft, :], in_=pf[:, k, 0:NQ],
                                func=Act.Relu,
                                bias=sm[:, C_B1 + ft:C_B1 + ft + 1])
                        else:
                            nc.vector.tensor_scalar(
                                out=hTb[:, ft, :], in0=pf[:, k, 0:NQ],
                                scalar1=sm[:, C_B1 + ft:C_B1 + ft + 1], scalar2=0.0,
                                op0=Alu.add, op1=Alu.max)
                emit_f2(FT)
                # bias chunk: ones x (b2 row) closes the accumulation
                for co in range(EC):
                    nc.tensor.matmul(out=p2s[co][:, 0:NQ],
                                     lhsT=w28[:, 16, PC * co:PC * (co + 1)],
                                     rhs=honk,
                                     start=False, stop=True)
                r = acts.tile([PC, EC, NQ], f32, tag="r", name=f"rf{l}", bufs=1)
                for co in range(EC):
                    nc.vector.scalar_tensor_tensor(
                        out=r[:, co, :], in0=p2s[co][:, 0:NQ],
                        scalar=1.0 / WS,
                        in1=tT[:, co, :], op0=Alu.mult, op1=Alu.add)
                tT, tb8, _ = layernorm(l, r, C_LN + 8, f"ln3_{l}")

            # ---- final LN + store ----
            outT, _, _ = layernorm(None, tT, None, "lnf", emit=False)
            nc.sync.dma_start(out=r2(d_out), in_=outT)

    nc.compile()
    return nc


def _pack_inputs(inputs, nlayers=L):
    """Host-side layout prep: transpose / cast / permute / scale / pack."""
    smalls = np.zeros((nlayers, PC, NS), np.float32)
    for l in range(nlayers):
        def put(col, vec):
            n = vec.shape[0] // PC
            smalls[l, :, col:col + n] = vec.reshape(n, PC).T
        put(C_BQ_SA, np.asarray(inputs["sa_bqkv"][l][:E], np.float32))
        # v-biases folded into the out-proj biases (softmax rows sum to 1)
        sa_bv = np.asarray(inputs["sa_bqkv"][l][2 * E:], np.float32)
        bo_sa = np.asarray(inputs["sa_bo"][l], np.float32) + \
            np.asarray(inputs["sa_wo"][l], np.float32) @ sa_bv
        put(C_BO_SA, bo_sa)
        put(C_BQ_CA, np.asarray(inputs["ca_bq"][l], np.float32))
        bo_ca = np.asarray(inputs["ca_bo"][l], np.float32) + \
            np.asarray(inputs["ca_wo"][l], np.float32) @ \
            np.asarray(inputs["ca_bv"][l], np.float32)
        put(C_BO_CA, bo_ca)
        put(C_B1, np.asarray(inputs["f_b1"][l], np.float32) * WS)
        put(C_LN, np.asarray(inputs["ln1g"][l], np.float32))
        put(C_LN + 2, np.asarray(inputs["ln1b"][l], np.float32))
        put(C_LN + 4, np.asarray(inputs["ln2g"][l], np.float32))
        put(C_LN + 6, np.asarray(inputs["ln2b"][l], np.float32))
        put(C_LN + 8, np.asarray(inputs["ln3g"][l], np.float32))
        put(C_LN + 10, np.asarray(inputs["ln3b"][l], np.float32))
    finals = np.zeros((PC, 4), np.float32)
    finals[:, 0:2] = np.asarray(inputs["lnfg"], np.float32).reshape(2, PC).T
    finals[:, 2:4] = np.asarray(inputs["lnfb"], np.float32).reshape(2, PC).T

    def T(x):
        return np.ascontiguousarray(np.swapaxes(np.asarray(x), -1, -2))

    wqkv = np.asarray(inputs["sa_wqkv"], np.float32)[:nlayers]
    w_sa_qk = wqkv[:, :2 * E]
    w_sa_v = wqkv[:, 2 * E:]
    w_ca_q = np.asarray(inputs["ca_wq"], np.float32)[:nlayers]
    w_ca_k = np.asarray(inputs["ca_wk"], np.float32)[:nlayers]

    # FFN1 fp8 with error-feedback halves [L, E, 2F]
    w1s = T(np.asarray(inputs["f_w1"], np.float32)[:nlayers]) * WS  # [L, E, F]
    w1a = w1s.astype(FP8)
    w1b = (w1s - w1a.astype(np.float32)).astype(FP8)
    w_f18 = np.concatenate([w1a, w1b], axis=2)
    # FFN2 bf16 + bias chunk: [L, 17*128, E]
    w2 = T(np.asarray(inputs["f_w2"], np.float32)[:nlayers])        # [L, F, E]
    w2x = np.zeros((nlayers, PC, E), np.float32)
    w2x[:, 0, :] = np.asarray(inputs["f_b2"], np.float32)[:nlayers] * WS
    w_f28 = np.concatenate([w2, w2x], axis=1)

    # identity for the PE mask-bias accumulate: key m = 64c + p
    # value 16: carries the x16 qw scale into the mask-bias accumulate
    ident = np.zeros((64, 2, PC), np.float32)
    for p in range(64):
        for c in range(2):
            ident[p, c, 64 * c + p] = 16.0

    shared = {
        "w_sa_qk8": (T(w_sa_qk) * WS).astype(FP8),
        "w_sa_v8": (T(w_sa_v) * WS).astype(FP8),
        "w_sa_o": T(inputs["sa_wo"][:nlayers]).astype(BF16),
        "w_ca_q8": (T(w_ca_q) * WS).astype(FP8),
        "w_ca_k8": (w_ca_k * WS).astype(FP8),
        "w_ca_v8": (T(np.asarray(inputs["ca_wv"], np.float32)[:nlayers]) * WS).astype(FP8),
        "w_ca_o": T(inputs["ca_wo"][:nlayers]).astype(BF16),
        "w_f18": w_f18,
        "w_f28": w_f28.astype(BF16),
        "smalls": smalls,
        "finals": finals,
        "ident8": ident.astype(FP8),
    }
    in_maps = []
    for b in range(B):
        m = dict(shared)
        m["tT"] = T(inputs["tgt"][b]).astype(np.float32)
        m["memT8"] = T(inputs["memory"][b]).astype(FP8)
        # mask bias [64, KT, 2, NQ]: key k = 128*kt + 64*c + p
        mk = np.asarray(inputs["geometry_mask"][b])       # [NQ, NK] bool
        mb = np.where(mk.T, 0.0, MASKB).astype(np.float32)  # [NK, NQ]
        m["maskb8"] = np.ascontiguousarray(
            mb.reshape(KT_CA, 2, 64, NQ).transpose(2, 0, 1, 3)).astype(FP8)
        # schraudolph bias+mask tile: key k = 128*kt + p
        bs = np.where(mk.T, B_SCH,
                      np.float32(BF16(B_SCH + A_SCH * SCALE * MASKB)))
        m["maskbs"] = np.ascontiguousarray(
            bs.reshape(KT_CA, PC, NQ).transpose(1, 0, 2)).astype(BF16)
        in_maps.append(m)
    return in_maps


_CACHE = {}


def kernel(run_opts=None, **inputs):
    nlayers = L
    if "nc" not in _CACHE:
        _CACHE["nc"] = build_nc(nlayers)
    nc = _CACHE["nc"]
    in_maps = _pack_inputs(inputs, nlayers)
    res = bass_utils.run_bass_kernel_spmd(
        nc, in_maps, core_ids=list(range(B)), **(run_opts or {}))
    _CACHE["last_result"] = res
    out = np.stack([np.asarray(r["outT"]).T for r in res.results])
    return np.ascontiguousarray(out.astype(np.float32))
